# revision 22
# baseline (speedup 1.0000x reference)
"""SANet-style attention (nn_Attention_1382979470038) on 8 TRN2 NeuronCores.

Sharding: 8 cores = 4 batches x 2 content-token halves (sequence parallel on
N, style tokens replicated within each pair).  No collectives needed: each
core computes output columns [C=512, N_loc=2048] of its batch independently.

Per-core math (b fixed, M = 4096 style tokens, N_loc = 2048 content tokens):
  instance-norm folded into conv weights:  F = (f_w . rstd_c) @ x_half + f_b'
  G  = (g_w . rstd_s) @ style + g_b'      [C, M]   (c on partitions)
  Ht = style^T @ h_w^T + h_b              [M, C]   (m on partitions)
  St = G^T F = scores transposed          [M, N]   (m on partitions)
  P  = exp(St)           (no max-subtraction: |S| <~ 30 is fp32-safe)
  den[n] = sum_m P[m,n]  (all-ones stationary matmul -> broadcast rows)
  styled = (Ht^T P) . (1/den)             [C, N]
  out = out_w @ styled + out_b            [C, N]

Matmul dtypes: float32r (FP22, full PE speed at free>=256) for convs+scores,
bf16 for the post-exp attention apply (benign precision there).
"""

import sys

sys.path.insert(0, "/opt/trn_rl_repo")

import numpy as np

import concourse.bass as bass
import concourse.tile as tile
from concourse import mybir

P = 128
C = 512
HW = 4096
NLOC = 2048
EPS = 1e-5
KT = C // P        # 4 k-tiles of 128 channels
NCH = NLOC // 512  # 4 n-chunks of 512
MCH = HW // 512    # 8 m-chunks of 512
MT = HW // P       # 32 m-tiles of 128

F32 = mybir.dt.float32
F32R = mybir.dt.float32r
BF16 = mybir.dt.bfloat16

AF = mybir.ActivationFunctionType
ALU = mybir.AluOpType


def _r(ap):
    return ap.bitcast(F32R)


def build_nc(hoist=True):
    nc = bass.Bass()
    cA = nc.declare_dram_parameter("cA", [C, NLOC], F32, isOutput=False)
    cB = nc.declare_dram_parameter("cB", [C, NLOC], F32, isOutput=False)
    style = nc.declare_dram_parameter("style", [C, HW], F32, isOutput=False)
    fwT = nc.declare_dram_parameter("fwT", [C, C], F32, isOutput=False)
    gwT = nc.declare_dram_parameter("gwT", [C, C], F32, isOutput=False)
    hwT = nc.declare_dram_parameter("hwT", [C, C], F32, isOutput=False)
    owT = nc.declare_dram_parameter("owT", [C, C], F32, isOutput=False)
    fb = nc.declare_dram_parameter("fb", [C, 1], F32, isOutput=False)
    gb = nc.declare_dram_parameter("gb", [C, 1], F32, isOutput=False)
    hb = nc.declare_dram_parameter("hb", [1, C], F32, isOutput=False)
    ob = nc.declare_dram_parameter("ob", [C, 1], F32, isOutput=False)
    ones = nc.declare_dram_parameter("ones", [1, P], F32, isOutput=False)
    out = nc.declare_dram_parameter("out", [C, NLOC], F32, isOutput=True)

    with tile.TileContext(nc) as tc:
        _build(tc, cA, cB, style, fwT, gwT, hwT, owT, fb, gb, hb, ob, ones,
               out)
    if hoist:
        _hoist_excess_waits(nc)
    return nc


# Walrus caps sync-wait commands per instruction (Activation/TensorScalar fit
# only one).  Hoist excess waits onto injected same-engine NOPs placed just
# before the instruction: engines execute in order, so semantics are identical.
_WAIT_EXEMPT = set()
_MAX_WAITS = 1  # NoOp/Drain chains carry up to this many per instruction


def _hoist_excess_waits(nc):
    ctr = [0]

    def mknop(engine, debug, waits, updates):
        ctr[0] += 1
        return mybir.InstNoOp(
            name=f"WH-{ctr[0]}", opcode="NoOp", engine=engine, debug=debug,
            ins=[], outs=[],
            sync_info=mybir.SyncInfo(on_wait=waits, on_update=updates),
        )

    for fn in nc.m.functions:
        for blk in fn.blocks:
            newl = []
            changed = False
            for inst in blk.instructions:
                si = getattr(inst, "sync_info", None)
                tn = type(inst).__name__
                eng = getattr(inst, "engine", None)
                cap = _MAX_WAITS if tn in ("InstNoOp", "InstDrain") else 1
                if (si is not None and tn not in _WAIT_EXEMPT
                        and si.on_wait and len(si.on_wait) > cap):
                    waits = list(si.on_wait)
                    keep, hoist = waits[-cap:], waits[:-cap]
                    for j in range(0, len(hoist), _MAX_WAITS):
                        newl.append(
                            mknop(eng, inst.debug, hoist[j:j + _MAX_WAITS], []))
                    inst.sync_info = mybir.SyncInfo(
                        on_wait=keep, on_update=list(si.on_update))
                    changed = True
                newl.append(inst)
            if changed:
                blk.instructions = newl


def _build(tc, cA, cB, style, fwT, gwT, hwT, owT, fb, gb, hb, ob, ones,
           out):
    nc = tc.nc
    from contextlib import ExitStack

    ctx = ExitStack()
    with ctx:
        # ---------- long-lived pools ----------
        small = ctx.enter_context(tc.tile_pool(name="small", bufs=32))
        cons = ctx.enter_context(tc.tile_pool(name="cons", bufs=1))
        fpool = ctx.enter_context(tc.tile_pool(name="fpool", bufs=4))
        owpool = ctx.enter_context(tc.tile_pool(name="owpool", bufs=4))
        # PSUM pools (8 banks total: 3 + 2 + 1 + 2)
        stps = ctx.enter_context(tc.tile_pool(name="stps", bufs=3, space="PSUM"))
        mmps = ctx.enter_context(tc.tile_pool(name="mmps", bufs=2, space="PSUM"))
        denps = ctx.enter_context(tc.tile_pool(name="denps", bufs=1, space="PSUM"))
        biasps = ctx.enter_context(tc.tile_pool(name="biasps", bufs=2, space="PSUM"))

        # constants
        ones_row = cons.tile([1, P], F32R, tag="ones_row")
        nc.gpsimd.dma_start(ones_row[:], ones[:, :].bitcast(F32R))
        ones_bf = cons.tile([P, P], BF16, tag="ones_bf")
        nc.vector.memset(ones_bf[:], 1.0)
        hb_s = cons.tile([1, C], F32R, tag="hb")
        nc.gpsimd.dma_start(hb_s[:], hb[:, :].bitcast(F32R))

        # per-partition bias vectors
        fb_s = [small.tile([P, 1], F32, tag="pb", name=f"fb{k}") for k in range(KT)]
        gb_s = [small.tile([P, 1], F32, tag="pb", name=f"gb{k}") for k in range(KT)]
        ob_s = [small.tile([P, 1], F32, tag="pb", name=f"ob{k}") for k in range(KT)]
        for k in range(KT):
            nc.gpsimd.dma_start(fb_s[k][:], fb[k * P:(k + 1) * P, :])
            nc.gpsimd.dma_start(gb_s[k][:], gb[k * P:(k + 1) * P, :])
            nc.gpsimd.dma_start(ob_s[k][:], ob[k * P:(k + 1) * P, :])

        F_sb = [fpool.tile([P, NLOC], F32R, tag="F", name=f"F{k}") for k in range(KT)]
        ow_s = [owpool.tile([P, C], F32R, tag="owT", name=f"ow{k}") for k in range(KT)]
        for k in range(KT):
            nc.gpsimd.dma_start(ow_s[k][:], owT[k * P:(k + 1) * P, :].bitcast(F32R))

        def load_w(pool, tag, src):
            ts = [pool.tile([P, C], F32R, tag=tag, name=f"{tag}{k}") for k in range(KT)]
            for k in range(KT):
                nc.gpsimd.dma_start(ts[k][:], src[k * P:(k + 1) * P, :].bitcast(F32R))
            return ts

        def stats_for(tiles_and_lens, scratch_pool, scr_cols):
            """tiles_and_lens: list of (sbuf_tile, ncols). Returns mean, rstd
            ([P,1] tiles) over the concatenation along the free axis."""
            total = sum(n for _, n in tiles_and_lens)
            sums, sqs = [], []
            for t, ncols in tiles_and_lens:
                s = small.tile([P, 1], F32, tag="stat")
                nc.vector.reduce_sum(s[:], t[:, :ncols].bitcast(F32), axis=mybir.AxisListType.X)
                sums.append(s)
                q = small.tile([P, 1], F32, tag="stat")
                scr = scratch_pool.tile([P, scr_cols], F32, tag="scr")
                nc.scalar.activation(scr[:, :ncols], t[:, :ncols].bitcast(F32),
                                     AF.Square)
                nc.vector.reduce_sum(q[:], scr[:, :ncols],
                                     axis=mybir.AxisListType.X)
                sqs.append(q)
            ssum = sums[0]
            for s in sums[1:]:
                t2 = small.tile([P, 1], F32, tag="stat")
                nc.vector.tensor_add(t2[:], ssum[:], s[:])
                ssum = t2
            qsum = sqs[0]
            for q in sqs[1:]:
                t2 = small.tile([P, 1], F32, tag="stat")
                nc.vector.tensor_add(t2[:], qsum[:], q[:])
                qsum = t2
            mean = small.tile([P, 1], F32R, tag="stat")
            nc.vector.tensor_scalar(mean[:], ssum[:], 1.0 / total, None,
                                    op0=ALU.mult)
            m2 = small.tile([P, 1], F32, tag="stat")
            nc.vector.tensor_mul(m2[:], mean[:].bitcast(F32), mean[:].bitcast(F32))
            v = small.tile([P, 1], F32, tag="stat")
            # v = qsum - total * m2
            nc.vector.scalar_tensor_tensor(
                out=v[:], in0=m2[:], scalar=-float(total), in1=qsum[:],
                op0=ALU.mult, op1=ALU.add)
            varp = small.tile([P, 1], F32, tag="stat")
            nc.vector.tensor_scalar(varp[:], v[:], 1.0 / (total - 1), EPS,
                                    op0=ALU.mult, op1=ALU.add)
            std = small.tile([P, 1], F32, tag="stat")
            nc.scalar.activation(std[:], varp[:], AF.Sqrt)
            rstd = small.tile([P, 1], F32, tag="stat")
            nc.vector.reciprocal(rstd[:], std[:])
            return mean, rstd

        def bias_fixup(w_tiles, mean_tiles, base_bias_tiles):
            """b'[o] = base[o] - sum_c w_scaled[c,o] * mean[c], per o-tile."""
            outb = []
            for j in range(KT):
                ps = biasps.tile([P, 1], F32, tag="biasps")
                for k in range(KT):
                    nc.tensor.matmul(
                        ps[:], w_tiles[k][:, j * P:(j + 1) * P].bitcast(F32),
                        mean_tiles[k][:].bitcast(F32),
                        start=(k == 0), stop=(k == KT - 1))
                bb = small.tile([P, 1], F32, tag="pb")
                nc.vector.tensor_sub(bb[:], base_bias_tiles[j][:], ps[:])
                outb.append(bb)
            return outb

        # ================= phase 1: content stats + F conv =================
        with tc.tile_pool(name="fwp", bufs=4) as fwp, \
             tc.tile_pool(name="cbig", bufs=4) as cbig, \
             tc.tile_pool(name="cstream", bufs=2) as cstream, \
             tc.tile_pool(name="scratch1", bufs=2) as scratch1:
            fw_raw = [fwp.tile([P, C], F32, tag="fwr", name=f"fwr{k}")
                      for k in range(KT)]
            for k in range(KT):
                nc.gpsimd.dma_start(fw_raw[k][:], fwT[k * P:(k + 1) * P, :])
            fw_s = [fwp.tile([P, C], F32R, tag="fws", name=f"fws{k}")
                    for k in range(KT)]
            cA_s = [cbig.tile([P, NLOC], F32R, tag="cA", name=f"cAs{k}") for k in range(KT)]
            cmean, crstd = [], []
            for k in range(KT):
                nc.sync.dma_start(cA_s[k][:], cA[k * P:(k + 1) * P, :].bitcast(F32R))
                cB_t = cstream.tile([P, NLOC], F32, tag="cB")
                nc.sync.dma_start(cB_t[:], cB[k * P:(k + 1) * P, :])
                mean, rstd = stats_for([(cA_s[k], NLOC), (cB_t, NLOC)],
                                       scratch1, NLOC)
                cmean.append(mean)
                crstd.append(rstd)
                # scale weight k-tile in place by rstd_c (per partition)
                nc.vector.tensor_scalar_mul(fw_s[k][:], fw_raw[k][:], crstd[k][:])
            fbp = bias_fixup(fw_s, cmean, fb_s)
            # F conv: F[o, n] over our half
            for ch in range(NCH):
                for j in range(KT):
                    ps = stps.tile([P, 512], F32, tag="stps")
                    for k in range(KT):
                        nc.tensor.matmul(
                            ps[:], _r(fw_s[k][:, j * P:(j + 1) * P]),
                            _r(cA_s[k][:, ch * 512:(ch + 1) * 512]),
                            start=(k == 0), stop=(k == KT - 1))
                    nc.scalar.activation(
                        F_sb[j][:, ch * 512:(ch + 1) * 512], ps[:],
                        AF.Identity, bias=fbp[j][:])

        # phase boundary: collapse recycled-zone deps into one sync point
        tc.strict_bb_all_engine_barrier()

        # ---------- pools for G / Ht (open after phase-1 space frees) ------
        gpool = ctx.enter_context(tc.tile_pool(name="gpool", bufs=4))
        htpool = ctx.enter_context(tc.tile_pool(name="htpool", bufs=1))
        G_sb = [gpool.tile([P, HW], F32R, tag="G", name=f"G{k}") for k in range(KT)]
        Ht_sb = htpool.tile([P, MT * C], BF16, tag="Ht")
        ghw_stack = ExitStack()
        ghwp = ghw_stack.enter_context(tc.tile_pool(name="ghwp", bufs=4))

        # ================= phase 2: style stats =================
        gw_raw = [ghwp.tile([P, C], F32, tag="gwr", name=f"gwr{k}")
                  for k in range(KT)]
        for k in range(KT):
            nc.gpsimd.dma_start(gw_raw[k][:], gwT[k * P:(k + 1) * P, :])
        gw_s = [ghwp.tile([P, C], F32R, tag="gws", name=f"gws{k}")
                for k in range(KT)]
        hw_s = load_w(ghwp, "hwT", hwT)
        with tc.tile_pool(name="sstat", bufs=2) as sstat, \
             tc.tile_pool(name="scratch2", bufs=2) as scratch2:
            smean, srstd = [], []
            for k in range(KT):
                st_a = sstat.tile([P, NLOC], F32, tag="sstat")
                nc.sync.dma_start(st_a[:], style[k * P:(k + 1) * P, :NLOC])
                st_b = sstat.tile([P, NLOC], F32, tag="sstat")
                nc.sync.dma_start(st_b[:], style[k * P:(k + 1) * P, NLOC:])
                mean, rstd = stats_for([(st_a, NLOC), (st_b, NLOC)],
                                       scratch2, NLOC)
                smean.append(mean)
                srstd.append(rstd)
                nc.vector.tensor_scalar_mul(gw_s[k][:], gw_raw[k][:], srstd[k][:])
            gbp = bias_fixup(gw_s, smean, gb_s)

        tc.strict_bb_all_engine_barrier()

        # ============ phase 3: G conv + Ht conv (style streamed) ========
        with tc.tile_pool(name="schunk", bufs=8) as schunk:
            for ch in range(MCH):
                sc = [schunk.tile([P, 512], F32R, tag="schunk", name=f"sc{ch}_{k}")
                      for k in range(KT)]
                for k in range(KT):
                    nc.sync.dma_start(
                        sc[k][:], style[k * P:(k + 1) * P,
                                        ch * 512:(ch + 1) * 512].bitcast(F32R))
                # G conv for this chunk of m
                for j in range(KT):
                    ps = stps.tile([P, 512], F32, tag="stps")
                    for k in range(KT):
                        nc.tensor.matmul(
                            ps[:], _r(gw_s[k][:, j * P:(j + 1) * P]),
                            _r(sc[k][:]),
                            start=(k == 0), stop=(k == KT - 1))
                    nc.scalar.activation(
                        G_sb[j][:, ch * 512:(ch + 1) * 512], ps[:],
                        AF.Identity, bias=gbp[j][:])
                # Ht conv: 4 m-tiles in this chunk
                for mi in range(4):
                    mt = ch * 4 + mi
                    ps = mmps.tile([P, 512], F32, tag="mmps")
                    for k in range(KT):
                        nc.tensor.matmul(
                            ps[:], _r(sc[k][:, mi * P:(mi + 1) * P]),
                            _r(hw_s[k][:]),
                            start=(k == 0), stop=False)
                    # bias via rank-1 ones row
                    nc.tensor.matmul(ps[:], _r(ones_row[:]), _r(hb_s[:]),
                                     start=False, stop=True)
                    nc.scalar.activation(
                        Ht_sb[:, mt * C:(mt + 1) * C], ps[:], AF.Copy)

        ghw_stack.close()
        tc.strict_bb_all_engine_barrier()

        # ================= phase 4: attention =================
        with tc.tile_pool(name="expp", bufs=1) as expp, \
             tc.tile_pool(name="styp", bufs=4) as styp, \
             tc.tile_pool(name="rdenp", bufs=1) as rdenp, \
             tc.tile_pool(name="outp", bufs=2) as outp:
            for ch in range(NCH):
                exp_t = expp.tile([P, MT * 512], BF16, tag="exp")
                den = denps.tile([P, 512], F32, tag="den")
                for mt in range(MT):
                    ps = stps.tile([P, 512], F32, tag="stps")
                    for k in range(KT):
                        nc.tensor.matmul(
                            ps[:], _r(G_sb[k][:, mt * P:(mt + 1) * P]),
                            _r(F_sb[k][:, ch * 512:(ch + 1) * 512]),
                            start=(k == 0), stop=(k == KT - 1))
                    nc.scalar.activation(
                        exp_t[:, mt * 512:(mt + 1) * 512], ps[:], AF.Exp)
                    nc.tensor.matmul(
                        den[:], ones_bf[:],
                        exp_t[:, mt * 512:(mt + 1) * 512],
                        start=(mt == 0), stop=(mt == MT - 1))
                rden = rdenp.tile([P, 512], F32, tag="rden")
                nc.vector.reciprocal(rden[:], den[:])
                # mm2 + scale by 1/den
                sty = []
                for j in range(KT):
                    ps = mmps.tile([P, 512], F32, tag="mmps")
                    for mt in range(MT):
                        nc.tensor.matmul(
                            ps[:],
                            Ht_sb[:, mt * C + j * P: mt * C + (j + 1) * P],
                            exp_t[:, mt * 512:(mt + 1) * 512],
                            start=(mt == 0), stop=(mt == MT - 1))
                    s_t = styp.tile([P, 512], F32R, tag="sty")
                    nc.vector.tensor_mul(s_t[:], ps[:], rden[:])
                    sty.append(s_t)
                # out conv
                for j in range(KT):
                    ps = mmps.tile([P, 512], F32, tag="mmps")
                    for k in range(KT):
                        nc.tensor.matmul(
                            ps[:], _r(ow_s[k][:, j * P:(j + 1) * P]),
                            _r(sty[k][:]),
                            start=(k == 0), stop=(k == KT - 1))
                    o_t = outp.tile([P, 512], F32, tag="outsb")
                    nc.scalar.activation(o_t[:], ps[:], AF.Identity,
                                         bias=ob_s[j][:])
                    nc.sync.dma_start(
                        out[j * P:(j + 1) * P, ch * 512:(ch + 1) * 512],
                        o_t[:])


_NC_CACHE = None


def _get_nc():
    global _NC_CACHE
    if _NC_CACHE is None:
        _NC_CACHE = build_nc()
    return _NC_CACHE


def kernel(content, style, f_w, f_b, g_w, g_b, h_w, h_b, out_w, out_b):
    from concourse.bass_utils import run_bass_kernel_spmd

    b, Cc, H, W = content.shape
    hw = H * W
    cf = np.ascontiguousarray(content.reshape(b, Cc, hw), dtype=np.float32)
    sf = np.ascontiguousarray(style.reshape(b, Cc, hw), dtype=np.float32)
    wT = {
        "fwT": np.ascontiguousarray(f_w.T, dtype=np.float32),
        "gwT": np.ascontiguousarray(g_w.T, dtype=np.float32),
        "hwT": np.ascontiguousarray(h_w.T, dtype=np.float32),
        "owT": np.ascontiguousarray(out_w.T, dtype=np.float32),
        "fb": np.ascontiguousarray(f_b.reshape(Cc, 1), dtype=np.float32),
        "gb": np.ascontiguousarray(g_b.reshape(Cc, 1), dtype=np.float32),
        "hb": np.ascontiguousarray(h_b.reshape(1, Cc), dtype=np.float32),
        "ob": np.ascontiguousarray(out_b.reshape(Cc, 1), dtype=np.float32),
        "ones": np.ones((1, P), dtype=np.float32),
    }
    in_maps = []
    for core in range(8):
        bi, hi = core // 2, core % 2
        in_maps.append({
            "cA": np.ascontiguousarray(cf[bi][:, hi * NLOC:(hi + 1) * NLOC]),
            "cB": np.ascontiguousarray(
                cf[bi][:, (1 - hi) * NLOC:(2 - hi) * NLOC]),
            "style": sf[bi],
            **wT,
        })
    global _LAST_IN_MAPS
    _LAST_IN_MAPS = in_maps
    nc = _get_nc()
    res = run_bass_kernel_spmd(nc, in_maps, core_ids=list(range(8)))
    outf = np.empty((b, Cc, hw), dtype=np.float32)
    for core in range(8):
        bi, hi = core // 2, core % 2
        outf[bi][:, hi * NLOC:(hi + 1) * NLOC] = res.results[core]["out"]
    return outf.reshape(b, Cc, H, W)
